# revision 12
# baseline (speedup 1.0000x reference)
"""Trainium2 Bass kernel for nn_Decoder_78237124264042.

6-layer causal decoder: V=32000, L=6, H=8, D=64, DM=512, DFF=1024, N=4, T=1024.

Sharding: 8 cores = 4 sequence-pairs. Pair {2i, 2i+1} handles sequence i with
tensor-parallel attention (4 heads per core); FFN + LayerNorm are replicated
within the pair so only one pair-AllGather (of the per-head attention outputs)
is needed per layer. Embedding gather runs on-device via dma_gather.

Compute in fp16 (PSUM accumulates fp32); softmax without max-subtraction
(logits are provably small for this model family).

Key structure choices:
- PV matmul emits TOKEN-major attention output (probabilities are the
  stationary operand), so the softmax denominator lands per-partition: the
  normalize is a [128,1] reciprocal + a per-partition-scaled scalar-engine
  copy, and the AllGather payload is already token-major (no per-head
  transposes, no 1-partition reciprocals, no partition broadcasts).
- The AllGather is split per head-pair so each collective overlaps the next
  pair's attention compute; a tiny warm-up collective at kernel start absorbs
  the first-collective rendezvous cost.
- All layer-constant bias vectors that hit the residual stream (V-projection
  bias + previous layer's folded LN2 beta) are merged into one cvec added on
  the GPSIMD engine off the critical path (broadcast-AP whole-tile ops); the
  g2/g1 residual affines also run on GPSIMD to keep the vector engine for LN.
- LN affines (g, b) are folded on the host into the downstream weight
  matrices (g2,b2 -> next layer's Wqkv; g1,b1 -> Wff; b1+bo -> residual bias)
  so the critical chain LN -> transpose -> matmul runs the bare normalize.
- Bounce-buffer writes and feature-major transposes are interleaved per
  token-half; next layer's weights are DMA'd mid-FFN2 off the critical path.
"""
import numpy as np
from contextlib import ExitStack

import concourse.bass as bass
import concourse.tile as tile
from concourse import bacc, mybir
from concourse.bass_utils import run_bass_kernel_spmd

V, L, H, D, DFF = 32000, 6, 8, 64, 1024
DM = H * D  # 512
N, T = 4, 1024
EPS = 1e-3
HC = H // 2          # heads per core
TC = T // 128        # token chunks (8)
KC = DM // 128       # dm chunks (4)
FC = DFF // 128      # dff chunks (8)
SCALE = 1.0 / np.sqrt(D)
F16 = mybir.dt.float16
F32 = mybir.dt.float32
AF = mybir.ActivationFunctionType


def _build(debug=False, no_cc=False):
    nc = bacc.Bacc(
        "TRN2",
        target_bir_lowering=False,
        debug=False,
        enable_asserts=True,
        num_devices=8,
    )

    def din(name, shape, dt=F16):
        return nc.dram_tensor(name, shape, dt, kind="ExternalInput").ap()

    emb = din("emb16", [V, DM])
    idxs = din("idxs", [128, T // 16], mybir.dt.int16)
    pos = din("pos", [128, TC, DM])
    wqk = din("wqk", [L, 128, KC, DM])       # g2[l-1]-folded for l>=1
    bqk = din("bqk", [L, 128, KC], F32)      # + b2[l-1] @ Wqk
    wv = din("wv", [L, 128, KC, HC * D])     # g2[l-1]-folded
    wff = din("wff", [L, 128, KC, DFF])      # g1[l]-folded
    bff = din("bff", [L, 128, FC], F32)      # + b1[l] @ Wff
    wo = din("wo", [L, 128, FC, DM])
    bob_rep = din("bob_rep", [L, 128, DM])   # bo[l] + b1[l], replicated
    cvec_rep = din("cvec_rep", [L, 128, DM])  # bv_full[l] + b2[l-1], replicated
    g1_rep = din("g1_rep", [L, 128, DM])
    g2_rep = din("g2_rep", [L, 128, DM])
    b2f_rep = din("b2f_rep", [128, DM])      # b2[L-1] for the final output
    diagm = din("diagm", [128, 128])         # binary keep-mask (s<=q), transposed

    out = nc.dram_tensor("out", [128, TC, DM], F32, kind="ExternalOutput").ap()

    def prb(name, tile_ap):
        if not debug:
            return
        t = nc.dram_tensor(f"prb_{name}", list(tile_ap.shape), tile_ap.dtype,
                           kind="ExternalOutput").ap()
        nc.sync.dma_start(t[:], tile_ap)

    with tile.TileContext(nc) as tc, ExitStack() as ctx:
        singles = ctx.enter_context(tc.tile_pool(name="singles", bufs=1))
        wpool = ctx.enter_context(tc.tile_pool(name="wpool", bufs=2))
        apool = ctx.enter_context(tc.tile_pool(name="apool", bufs=1))
        ypool = ctx.enter_context(tc.tile_pool(name="ypool", bufs=2))
        ppool = ctx.enter_context(tc.tile_pool(name="ppool", bufs=3))
        tpool = ctx.enter_context(tc.tile_pool(name="tpool", bufs=2))
        psum_mm = ctx.enter_context(tc.tile_pool(name="psum_mm", bufs=2, space="PSUM"))
        psum_lg = ctx.enter_context(tc.tile_pool(name="psum_lg", bufs=2, space="PSUM"))
        psum_pv = ctx.enter_context(tc.tile_pool(name="psum_pv", bufs=2, space="PSUM"))
        dram = ctx.enter_context(tc.tile_pool(name="dram", bufs=2, space="DRAM"))

        # --- persistent tiles ---
        h = singles.tile([128, TC, DM], F16)   # token-major residual master
        pos_sb = singles.tile([128, TC, DM], F16)
        idxs_sb = singles.tile([128, T // 16], mybir.dt.int16)
        diag_sb = singles.tile([128, 128], F16)
        eps_sb = singles.tile([128, 1], F32)
        nc.vector.memset(eps_sb[:], EPS)
        nc.sync.dma_start(pos_sb[:], pos[:])
        idx_load = nc.sync.dma_start(idxs_sb[:], idxs[:])
        nc.sync.dma_start(diag_sb[:], diagm[:])

        hT = singles.tile([128, KC, T], F16)
        h1T = singles.tile([128, KC, T], F16)

        # warm-up collective: absorbs the first-collective rendezvous cost
        # while the embedding gather runs.
        wua = dram.tile([128, 16], F16, tag="wua")
        wuo = dram.tile([256, 16], F16, tag="wuo")
        nc.sync.dma_start(wua[:], diag_sb[:, 0:16])
        if not no_cc:
            nc.gpsimd.collective_compute(
                "AllGather", mybir.AluOpType.bypass,
                replica_groups=[[0, 1], [2, 3], [4, 5], [6, 7]],
                ins=[wua[:].opt()], outs=[wuo[:].opt()],
            )

        # --- embedding gather: h[p, c, :] = emb16[ids[c*128+p], :] ---
        gat = nc.gpsimd.dma_gather(
            h[:], emb[:], idxs_sb[:],
            num_idxs=T, num_idxs_reg=T, elem_size=DM, elem_step=DM,
        )
        tile.add_dep_helper(gat.ins, idx_load.ins, reason="gather reads idxs_sb")
        nc.vector.tensor_add(h[:], h[:], pos_sb[:])
        prb("h0", h[:])

        # hd: token-major DRAM bounce feeding the feature-major transposes.
        hd = dram.tile([TC, 128, DM], F16, tag="hd")
        for t in range(TC):
            nc.sync.dma_start(hd[t], h[:, t, :])

        trsrc = h  # input of the next QKV (pre-affine); layer 0: h itself

        def load_weights(l, eng):
            shapes = {
                "wqk": ([128, KC, DM], F16), "wv": ([128, KC, HC * D], F16),
                "wff": ([128, KC, DFF], F16), "wo": ([128, FC, DM], F16),
                "bqk": ([128, KC], F32), "bff": ([128, FC], F32),
                "bob": ([128, DM], F16), "cvec": ([128, DM], F16),
                "g1": ([128, DM], F16), "g2": ([128, DM], F16),
            }
            w = {k: wpool.tile(s, dt, tag=k, name=f"w_{k}")
                 for k, (s, dt) in shapes.items()}
            for name, src_t in [("wqk", wqk), ("wv", wv), ("wff", wff),
                                ("wo", wo), ("bqk", bqk), ("bff", bff)]:
                eng.dma_start(w[name][:], src_t[l])
            eng.dma_start(w["bob"][:], bob_rep[l])
            eng.dma_start(w["cvec"][:], cvec_rep[l])
            eng.dma_start(w["g1"][:], g1_rep[l])
            eng.dma_start(w["g2"][:], g2_rep[l])
            return w

        def hT_transposes(dst, src_d, nh, eng):
            for k in range(KC):
                eng.dma_start_transpose(
                    dst[:, k, nh * 512:(nh + 1) * 512],
                    src_d[nh * 4:(nh + 1) * 4, :, k * 128:(k + 1) * 128]
                    .rearrange("c p d -> (c p) d"),
                )

        wnext = load_weights(0, nc.sync)
        # layer 0 feature-major input (layers >= 1 do this in the FFN2 loop)
        for nh in range(2):
            hT_transposes(hT, hd, nh, nc.sync)

        for l in range(L):
            w = wnext
            wqk_sb, wv_sb, wff_sb, wo_sb = w["wqk"], w["wv"], w["wff"], w["wo"]
            bqk_sb, bff_sb, bob_sb = w["bqk"], w["bff"], w["bob"]
            cvec_sb, g1_sb, g2_sb = w["cvec"], w["g1"], w["g2"]
            if l == 0:
                prb("hT0", hT[:])

            # --- qkT = WqkT @ h : rows [q(4 heads)|k(4 heads)], cols T ---
            qk_sb = apool.tile([128, KC, T], F16, tag="qk")
            for n in range(2):
                for m in range(4):
                    ps = psum_mm.tile([128, 512], F32, tag="mm")
                    for k in range(KC):
                        nc.tensor.matmul(
                            ps[:],
                            wqk_sb[:, k, m * 128:(m + 1) * 128],
                            hT[:, k, n * 512:(n + 1) * 512],
                            start=(k == 0), stop=(k == KC - 1),
                        )
                    dst = qk_sb[:, m, n * 512:(n + 1) * 512]
                    if m % 2 == 0:
                        nc.scalar.activation(dst, ps[:], AF.Identity,
                                             bias=bqk_sb[:, m:m + 1])
                    else:
                        nc.vector.tensor_scalar(
                            dst, ps[:], bqk_sb[:, m:m + 1], None,
                            mybir.AluOpType.add)

            # --- v (token-major, with ones column for the softmax denom) ---
            v_sb = apool.tile([128, TC, HC, D + 1], F16, tag="v")
            nc.vector.memset(v_sb[:, :, :, D:D + 1], 1.0)
            for t in range(TC):
                ps = psum_mm.tile([128, HC * D], F32, tag="mm")
                for k in range(KC):
                    nc.tensor.matmul(
                        ps[:],
                        hT[:, k, t * 128:(t + 1) * 128],
                        wv_sb[:, k, :],
                        start=(k == 0), stop=(k == KC - 1),
                    )
                nc.vector.tensor_copy(
                    v_sb[:, t, :, 0:D],
                    ps[:].rearrange("p (h d) -> p h d", h=HC),
                )

            # --- residual base (GPSIMD, off critical path):
            # x = y2*g2 + cvec   (cvec = V-bias + b2[l-1]; l=0: x = h + cvec)
            g2b = g2_sb[:].rearrange("p d -> p () d").broadcast_to([128, TC, DM])
            cvb = cvec_sb[:].rearrange("p d -> p () d").broadcast_to([128, TC, DM])
            if l > 0:
                nc.gpsimd.tensor_mul(h[:], trsrc[:], g2b)
            nc.gpsimd.tensor_add(h[:], h[:], cvb)

            # --- attention (4 local heads, by pairs), token-major output ---
            # After each head pair, its 128 output columns are AllGather-ed so
            # the collective overlaps the next pair's compute.
            a_all = apool.tile([128, TC, HC * D], F16, tag="a_all")
            a_tok = apool.tile([128, TC, DM], F16, tag="a_tok")
            for pair in range(2):
                for hh in (2 * pair, 2 * pair + 1):
                    qT = qk_sb[64 * (hh % 2):64 * (hh % 2) + 64, hh // 2, :]
                    kT = qk_sb[64 * (hh % 2):64 * (hh % 2) + 64, 2 + hh // 2, :]
                    # phase 1: pT[si] = exp(scale * K_si^T Q), diag-masked
                    pts = []
                    for si in range(TC):
                        q0 = si * 128
                        lg = psum_lg.tile([128, T], F32, tag="lg")
                        if q0 < 512:
                            nc.tensor.matmul(lg[:, q0:512], kT[:, q0:q0 + 128],
                                             qT[:, q0:512], start=True, stop=True)
                            nc.tensor.matmul(lg[:, 512:1024], kT[:, q0:q0 + 128],
                                             qT[:, 512:1024], start=True, stop=True)
                        else:
                            nc.tensor.matmul(lg[:, q0:1024], kT[:, q0:q0 + 128],
                                             qT[:, q0:1024], start=True, stop=True)
                        pT = tpool.tile([128, T], F16, tag=f"pT{si}")
                        cols = T - q0
                        nc.scalar.activation(pT[:, 0:cols], lg[:, q0:T], AF.Exp,
                                             scale=float(SCALE))
                        nc.vector.tensor_mul(pT[:, 0:128], pT[:, 0:128], diag_sb[:])
                        pts.append(pT)
                    # phase 2: token-major PV; col D is the softmax denominator
                    for qi in range(TC):
                        pv = psum_pv.tile([128, D + 1], F32, tag="pv")
                        for si in range(qi + 1):
                            off = (qi - si) * 128
                            nc.tensor.matmul(
                                pv[:],
                                pts[si][:, off:off + 128],
                                v_sb[:, si, hh, :],
                                start=(si == 0), stop=(si == qi),
                            )
                        rden = ppool.tile([128, 1], F32, tag="rden")
                        nc.vector.reciprocal(rden[:], pv[:, D:D + 1])
                        nc.scalar.activation(
                            a_all[:, qi, hh * D:(hh + 1) * D], pv[:, 0:D],
                            AF.Identity, scale=rden[:, 0:1])
                # AllGather this pair's 128 columns; assemble + accumulate
                c0, c1 = pair * 128, pair * 128 + 128
                agi = dram.tile([128, TC, 128], F16, tag=f"agi{pair}")
                ago = dram.tile([256, TC, 128], F16, tag=f"ago{pair}")
                nc.sync.dma_start(agi[:], a_all[:, :, c0:c1])
                if no_cc:
                    nc.sync.dma_start(ago[0:128], agi[:])
                    nc.sync.dma_start(ago[128:256], agi[:])
                else:
                    nc.gpsimd.collective_compute(
                        "AllGather", mybir.AluOpType.bypass,
                        replica_groups=[[0, 1], [2, 3], [4, 5], [6, 7]],
                        ins=[agi[:].opt()], outs=[ago[:].opt()],
                    )
                nc.sync.dma_start(a_tok[:, :, c0:c1], ago[0:128])
                nc.sync.dma_start(a_tok[:, :, 256 + c0:256 + c1], ago[128:256])
                adde = nc.gpsimd if pair == 0 else nc.vector
                adde.tensor_add(
                    h[:, :, c0:c1], h[:, :, c0:c1], a_tok[:, :, c0:c1])
                adde.tensor_add(
                    h[:, :, 256 + c0:256 + c1], h[:, :, 256 + c0:256 + c1],
                    a_tok[:, :, 256 + c0:256 + c1])
            if l == 0:
                prb("a0", a_all[:])

            # keep-warm: dependency-free matmuls into a write-only psum
            # scratch; they run during the AG2/LN1 window (PE otherwise idle
            # >3.4us would re-throttle to 1.2 GHz) and drain before FFN1's
            # inputs are ready.
            warm = psum_lg.tile([128, T], F32, tag="lg")
            for j in range(72):
                nc.tensor.matmul(warm[:, 0:512],
                                 wqk_sb[:, 0, 0:128], hT[:, 0, 0:512],
                                 start=True, stop=True)

            # --- LN1 -> y1; h1d bounce; x2base on GPSIMD (per half) ---
            y1 = ypool.tile([128, TC, DM], F16, tag="y")
            h1d = dram.tile([TC, 128, DM], F16, tag="h1d")
            g1b = g1_sb[:].rearrange("p d -> p () d").broadcast_to([128, 4, DM])
            bobb = bob_sb[:].rearrange("p d -> p () d").broadcast_to([128, 4, DM])
            for half in range(2):
                t0, t1 = 4 * half, 4 * half + 4
                for t in range(t0, t1):
                    _ln_chunk(nc, ppool, h, y1, t, eps_sb)
                    nc.sync.dma_start(h1d[t], y1[:, t, :])
                hT_transposes(h1T, h1d, half, nc.sync)
                # x2base = y1*g1 + (b1+bo), off critical path
                nc.gpsimd.tensor_mul(h[:, t0:t1, :], y1[:, t0:t1, :], g1b)
                nc.gpsimd.tensor_add(h[:, t0:t1, :], h[:, t0:t1, :], bobb)

            # --- FFN: ffT = relu(Wff'T @ y1 + bff'); o = ffT.T @ Wo ---
            ff_sb = apool.tile([128, FC, T], F16, tag="ff")
            for n in range(2):
                for m in range(FC):
                    ps = psum_mm.tile([128, 512], F32, tag="mm")
                    for k in range(KC):
                        nc.tensor.matmul(
                            ps[:],
                            wff_sb[:, k, m * 128:(m + 1) * 128],
                            h1T[:, k, n * 512:(n + 1) * 512],
                            start=(k == 0), stop=(k == KC - 1),
                        )
                    dst = ff_sb[:, m, n * 512:(n + 1) * 512]
                    if m % 2 == 0:
                        nc.scalar.activation(dst, ps[:], AF.Relu,
                                             bias=bff_sb[:, m:m + 1])
                    else:
                        nc.vector.tensor_scalar(
                            dst, ps[:], bff_sb[:, m:m + 1], 0.0,
                            mybir.AluOpType.add, mybir.AluOpType.max)

            # --- FFN2 + residual + LN2; hd bounce for next layer ---
            y2 = ypool.tile([128, TC, DM], F16, tag="y")
            for t in range(TC):
                ps = psum_mm.tile([128, DM], F32, tag="mm")
                for k in range(FC):
                    nc.tensor.matmul(
                        ps[:],
                        ff_sb[:, k, t * 128:(t + 1) * 128],
                        wo_sb[:, k, :],
                        start=(k == 0), stop=(k == FC - 1),
                    )
                nc.vector.tensor_add(h[:, t, :], h[:, t, :], ps[:])
                _ln_chunk(nc, ppool, h, y2, t, eps_sb)
                if l < L - 1:
                    nc.sync.dma_start(hd[t], y2[:, t, :])
                    if t == 3 or t == 7:
                        hT_transposes(hT, hd, t // 4, nc.sync)
                    if t == 3:
                        wnext = load_weights(l + 1, nc.sync)
            if l < L - 1:
                warm2 = psum_lg.tile([128, T], F32, tag="lg")
                for j in range(20):
                    nc.tensor.matmul(warm2[:, 0:512],
                                     wo_sb[:, 0, 0:128], ff_sb[:, 0, 0:512],
                                     start=True, stop=True)
            trsrc = y2

        # --- output: h_final = y2*g2 + b2, cast to f32 ---
        b2f_sb = singles.tile([128, DM], F16)
        nc.sync.dma_start(b2f_sb[:], b2f_rep[:])
        ho = singles.tile([128, TC, DM], F32)
        for t in range(TC):
            nc.vector.tensor_mul(h[:, t, :], trsrc[:, t, :], g2_sb[:])
            nc.vector.tensor_add(h[:, t, :], h[:, t, :], b2f_sb[:])
            nc.scalar.copy(ho[:, t, :], h[:, t, :])
            nc.sync.dma_start(out[:, t, :], ho[:, t, :])

    nc.finalize()
    return nc


def _ln_chunk(nc, pool, x, y, t, eps_sb):
    """LayerNorm (no affine) of chunk t: y[:, t, :] = (x_t - mean)/std."""
    stats = pool.tile([128, TC, 6], F32, tag="ln_stats")
    mv = pool.tile([128, TC, 2], F32, tag="ln_mv")
    rstd = pool.tile([128, TC], F32, tag="ln_rstd")
    nc.vector.bn_stats(stats[:, t, :], x[:, t, :])
    nc.vector.bn_aggr(mv[:, t, :], stats[:, t, :])
    nc.scalar.activation(rstd[:, t:t + 1], mv[:, t, 1:2], AF.Sqrt, bias=eps_sb[:])
    nc.vector.reciprocal(rstd[:, t:t + 1], rstd[:, t:t + 1])
    nc.vector.tensor_scalar(
        y[:, t, :], x[:, t, :],
        mv[:, t, 0:1], rstd[:, t:t + 1],
        mybir.AluOpType.subtract, mybir.AluOpType.mult,
    )


_NC_CACHE = {}


def _get_nc(**kw):
    key = tuple(sorted(kw.items()))
    if key not in _NC_CACHE:
        _NC_CACHE[key] = _build(**kw)
    return _NC_CACHE[key]


def _prep_inputs(x, emb, Wqkv, bqkv, Wff, bff, Wo, bo, g1, beta1, g2, beta2):
    """Host-side sharding + LN-affine folding: build the 8 per-core maps."""
    f16 = np.float16
    f32 = np.float32
    emb16 = np.ascontiguousarray((np.asarray(emb) * np.sqrt(f32(DM))).astype(f16))

    p_ = np.arange(T, dtype=f32)[:, None]
    i_ = np.arange(DM, dtype=f32)[None, :]
    rates = 1.0 / np.power(10000.0, 2.0 * np.floor(i_ / 2.0) / DM)
    ang = p_ * rates
    even = (np.arange(DM) % 2) == 0
    pos = np.where(even[None, :], np.sin(ang), np.cos(ang)).astype(f16)
    pos_l = np.ascontiguousarray(pos.reshape(TC, 128, DM).transpose(1, 0, 2))

    Wqkv = np.asarray(Wqkv, f32)
    bqkv = np.asarray(bqkv, f32)
    Wff_ = np.asarray(Wff, f32)
    Wo_ = np.asarray(Wo, f32)
    bff_ = np.asarray(bff, f32)
    bo_ = np.asarray(bo, f32)
    g1_ = np.asarray(g1, f32)
    b1_ = np.asarray(beta1, f32)
    g2_ = np.asarray(g2, f32)
    b2_ = np.asarray(beta2, f32)

    # fold g2[l-1], b2[l-1] into layer l's QKV weights (l >= 1)
    gprev = np.ones((L, DM), f32)
    bprev = np.zeros((L, DM), f32)
    gprev[1:] = g2_[:-1]
    bprev[1:] = b2_[:-1]
    Wqkv_f = Wqkv * gprev[:, :, None]
    bqkv_f = bqkv + np.einsum("ld,ldc->lc", bprev, Wqkv)
    # fold g1[l], b1[l] into Wff
    Wff_f = Wff_ * g1_[:, :, None]
    bff_f = bff_ + np.einsum("ld,ldc->lc", b1_, Wff_)
    bob = (bo_ + b1_).astype(f16)

    Wh = Wqkv_f.reshape(L, DM, H, D, 3)
    bh = bqkv_f.reshape(L, H, D, 3)

    def dm_part(w):  # [L, DM, C] -> [L, 128, KC, C]
        Lx, dm, C = w.shape
        return np.ascontiguousarray(
            w.reshape(Lx, dm // 128, 128, C).transpose(0, 2, 1, 3))

    wff_l = dm_part(Wff_f).astype(f16)
    wo_l = dm_part(Wo_).astype(f16)
    bff_l = np.ascontiguousarray(bff_f.reshape(L, FC, 128).transpose(0, 2, 1))

    def rep(v):  # [L, DM] -> [L, 128, DM] replicated f16
        return np.ascontiguousarray(np.broadcast_to(
            np.asarray(v, f16)[:, None, :], (L, 128, DM)))

    # cvec = V-projection bias (folded) + b2[l-1]: hits the residual directly
    bv_full = bh[:, :, :, 2].reshape(L, DM)
    cvec = bv_full + bprev

    bob_l = rep(bob)
    cvec_l = rep(cvec)
    g1_l = rep(g1_)
    g2_l = rep(g2_)
    b2f = np.ascontiguousarray(
        np.broadcast_to(b2_[L - 1].astype(f16)[None, :], (128, DM)))

    s_i = np.arange(128)[:, None]
    q_i = np.arange(128)[None, :]
    diag = (s_i <= q_i).astype(f16)

    x = np.asarray(x)
    in_maps = []
    for c in range(8):
        seq, half = c // 2, c % 2
        hs = slice(half * HC, half * HC + HC)
        wq = Wh[:, :, hs, :, 0].reshape(L, DM, HC * D)
        wk = Wh[:, :, hs, :, 1].reshape(L, DM, HC * D)
        wqk_c = dm_part(np.concatenate([wq, wk], axis=2)).astype(f16)
        bq = bh[:, hs, :, 0].reshape(L, HC * D)
        bk = bh[:, hs, :, 1].reshape(L, HC * D)
        bqk_c = np.ascontiguousarray(
            np.concatenate([bq, bk], 1).reshape(L, KC, 128).transpose(0, 2, 1)
        ).astype(f32)
        wv_c = dm_part(Wh[:, :, hs, :, 2].reshape(L, DM, HC * D)).astype(f16)

        ids = np.asarray(x[seq], np.int64)
        idx_w = np.ascontiguousarray(
            np.tile(ids.reshape(T // 16, 16).T.astype(np.int16), (8, 1)))

        in_maps.append({
            "emb16": emb16, "idxs": idx_w, "pos": pos_l,
            "wqk": wqk_c, "bqk": bqk_c, "wv": wv_c,
            "wff": wff_l, "bff": bff_l, "wo": wo_l, "bob_rep": bob_l,
            "cvec_rep": cvec_l,
            "g1_rep": g1_l, "g2_rep": g2_l, "b2f_rep": b2f,
            "diagm": diag,
        })
    return in_maps


def kernel(**inputs) -> np.ndarray:
    nc = _get_nc()
    in_maps = _prep_inputs(**inputs)
    res = run_bass_kernel_spmd(nc, in_maps, core_ids=list(range(8)))
    outs = []
    for seq in range(N):
        o = res.results[2 * seq]["out"]  # [128, TC, DM], token t = c*128+p
        outs.append(o.transpose(1, 0, 2).reshape(T, DM))
    return np.stack(outs).astype(np.float32)


# revision 14
# speedup vs baseline: 1.0054x; 1.0054x over previous
"""Trainium2 Bass kernel for nn_Decoder_78237124264042.

6-layer causal decoder: V=32000, L=6, H=8, D=64, DM=512, DFF=1024, N=4, T=1024.

Sharding: 8 cores = 4 sequence-pairs. Pair {2i, 2i+1} handles sequence i with
tensor-parallel attention (4 heads per core); FFN + LayerNorm are replicated
within the pair so only one pair-AllGather (of the per-head attention outputs)
is needed per layer. Embedding gather runs on-device via dma_gather.

Compute in fp16 (PSUM accumulates fp32); softmax without max-subtraction
(logits are provably small for this model family).

Key structure choices:
- PV matmul emits TOKEN-major attention output (probabilities are the
  stationary operand), so the softmax denominator lands per-partition: the
  normalize is a [128,1] reciprocal + a per-partition-scaled scalar-engine
  copy, and the AllGather payload is already token-major (no per-head
  transposes, no 1-partition reciprocals, no partition broadcasts).
- The AllGather is split per head-pair so each collective overlaps the next
  pair's attention compute; a tiny warm-up collective at kernel start absorbs
  the first-collective rendezvous cost.
- All layer-constant bias vectors that hit the residual stream (V-projection
  bias + previous layer's folded LN2 beta) are merged into one cvec added on
  the GPSIMD engine off the critical path (broadcast-AP whole-tile ops); the
  g2/g1 residual affines also run on GPSIMD to keep the vector engine for LN.
- LN affines (g, b) are folded on the host into the downstream weight
  matrices (g2,b2 -> next layer's Wqkv; g1,b1 -> Wff; b1+bo -> residual bias)
  so the critical chain LN -> transpose -> matmul runs the bare normalize.
- Bounce-buffer writes and feature-major transposes are interleaved per
  token-half; next layer's weights are DMA'd mid-FFN2 off the critical path.
"""
import numpy as np
from contextlib import ExitStack

import concourse.bass as bass
import concourse.tile as tile
from concourse import bacc, mybir
from concourse.bass_utils import run_bass_kernel_spmd

V, L, H, D, DFF = 32000, 6, 8, 64, 1024
DM = H * D  # 512
N, T = 4, 1024
EPS = 1e-3
HC = H // 2          # heads per core
TC = T // 128        # token chunks (8)
KC = DM // 128       # dm chunks (4)
FC = DFF // 128      # dff chunks (8)
SCALE = 1.0 / np.sqrt(D)
F16 = mybir.dt.float16
F32 = mybir.dt.float32
AF = mybir.ActivationFunctionType


def _build(debug=False, no_cc=False):
    nc = bacc.Bacc(
        "TRN2",
        target_bir_lowering=False,
        debug=False,
        enable_asserts=True,
        num_devices=8,
    )

    def din(name, shape, dt=F16):
        return nc.dram_tensor(name, shape, dt, kind="ExternalInput").ap()

    emb = din("emb16", [V, DM])
    idxs = din("idxs", [128, T // 16], mybir.dt.int16)
    pos = din("pos", [128, TC, DM])
    wqk = din("wqk", [L, 128, KC, DM])       # g2[l-1]-folded for l>=1
    bqk = din("bqk", [L, 128, KC], F32)      # + b2[l-1] @ Wqk
    wv = din("wv", [L, 128, KC, HC * D])     # g2[l-1]-folded
    wff = din("wff", [L, 128, KC, DFF])      # g1[l]-folded
    bff = din("bff", [L, 128, FC], F32)      # + b1[l] @ Wff
    wo = din("wo", [L, 128, FC, DM])
    bob_rep = din("bob_rep", [L, 128, DM])   # bo[l] + b1[l], replicated
    cvec_rep = din("cvec_rep", [L, 128, DM])  # bv_full[l] + b2[l-1], replicated
    g1_rep = din("g1_rep", [L, 128, DM])
    g2_rep = din("g2_rep", [L, 128, DM])
    b2f_rep = din("b2f_rep", [128, DM])      # b2[L-1] for the final output
    diagm = din("diagm", [128, 128])         # binary keep-mask (s<=q), transposed

    out = nc.dram_tensor("out", [128, TC, DM], F32, kind="ExternalOutput").ap()

    def prb(name, tile_ap):
        if not debug:
            return
        t = nc.dram_tensor(f"prb_{name}", list(tile_ap.shape), tile_ap.dtype,
                           kind="ExternalOutput").ap()
        nc.sync.dma_start(t[:], tile_ap)

    with tile.TileContext(nc) as tc, ExitStack() as ctx:
        singles = ctx.enter_context(tc.tile_pool(name="singles", bufs=1))
        wpool = ctx.enter_context(tc.tile_pool(name="wpool", bufs=2))
        apool = ctx.enter_context(tc.tile_pool(name="apool", bufs=1))
        ypool = ctx.enter_context(tc.tile_pool(name="ypool", bufs=2))
        ppool = ctx.enter_context(tc.tile_pool(name="ppool", bufs=8))
        tpool = ctx.enter_context(tc.tile_pool(name="tpool", bufs=2))
        psum_mm = ctx.enter_context(tc.tile_pool(name="psum_mm", bufs=2, space="PSUM"))
        psum_lg = ctx.enter_context(tc.tile_pool(name="psum_lg", bufs=2, space="PSUM"))
        psum_pv = ctx.enter_context(tc.tile_pool(name="psum_pv", bufs=2, space="PSUM"))
        dram = ctx.enter_context(tc.tile_pool(name="dram", bufs=2, space="DRAM"))

        # --- persistent tiles ---
        h = singles.tile([128, TC, DM], F16)   # token-major residual master
        pos_sb = singles.tile([128, TC, DM], F16)
        idxs_sb = singles.tile([128, T // 16], mybir.dt.int16)
        diag_sb = singles.tile([128, 128], F16)
        eps_sb = singles.tile([128, 1], F32)
        nc.vector.memset(eps_sb[:], EPS)
        nc.sync.dma_start(pos_sb[:], pos[:])
        idx_load = nc.sync.dma_start(idxs_sb[:], idxs[:])
        nc.sync.dma_start(diag_sb[:], diagm[:])

        hT = singles.tile([128, KC, T], F16)
        h1T = singles.tile([128, KC, T], F16)

        # warm-up collective: absorbs the first-collective rendezvous cost
        # while the embedding gather runs.
        wua = dram.tile([128, 16], F16, tag="wua")
        wuo = dram.tile([256, 16], F16, tag="wuo")
        nc.sync.dma_start(wua[:], diag_sb[:, 0:16])
        if not no_cc:
            nc.gpsimd.collective_compute(
                "AllGather", mybir.AluOpType.bypass,
                replica_groups=[[0, 1], [2, 3], [4, 5], [6, 7]],
                ins=[wua[:].opt()], outs=[wuo[:].opt()],
            )

        # --- embedding gather: h[p, c, :] = emb16[ids[c*128+p], :] ---
        gat = nc.gpsimd.dma_gather(
            h[:], emb[:], idxs_sb[:],
            num_idxs=T, num_idxs_reg=T, elem_size=DM, elem_step=DM,
        )
        tile.add_dep_helper(gat.ins, idx_load.ins, reason="gather reads idxs_sb")
        nc.vector.tensor_add(h[:], h[:], pos_sb[:])
        prb("h0", h[:])

        # hd: token-major DRAM bounce feeding the feature-major transposes.
        hd = dram.tile([TC, 128, DM], F16, tag="hd")
        for t in range(TC):
            nc.sync.dma_start(hd[t], h[:, t, :])

        trsrc = h  # input of the next QKV (pre-affine); layer 0: h itself

        def load_weights(l, eng):
            shapes = {
                "wqk": ([128, KC, DM], F16), "wv": ([128, KC, HC * D], F16),
                "wff": ([128, KC, DFF], F16), "wo": ([128, FC, DM], F16),
                "bqk": ([128, KC], F32), "bff": ([128, FC], F32),
                "bob": ([128, DM], F16), "cvec": ([128, DM], F16),
                "g1": ([128, DM], F16), "g2": ([128, DM], F16),
            }
            w = {k: wpool.tile(s, dt, tag=k, name=f"w_{k}")
                 for k, (s, dt) in shapes.items()}
            for name, src_t in [("wqk", wqk), ("wv", wv), ("wff", wff),
                                ("wo", wo), ("bqk", bqk), ("bff", bff)]:
                eng.dma_start(w[name][:], src_t[l])
            eng.dma_start(w["bob"][:], bob_rep[l])
            eng.dma_start(w["cvec"][:], cvec_rep[l])
            eng.dma_start(w["g1"][:], g1_rep[l])
            eng.dma_start(w["g2"][:], g2_rep[l])
            return w

        def hT_transposes(dst, src_d, nh, eng):
            for k in range(KC):
                eng.dma_start_transpose(
                    dst[:, k, nh * 512:(nh + 1) * 512],
                    src_d[nh * 4:(nh + 1) * 4, :, k * 128:(k + 1) * 128]
                    .rearrange("c p d -> (c p) d"),
                )

        wnext = load_weights(0, nc.sync)
        # layer 0 feature-major input (layers >= 1 do this in the FFN2 loop)
        for nh in range(2):
            hT_transposes(hT, hd, nh, nc.sync)

        for l in range(L):
            w = wnext
            wqk_sb, wv_sb, wff_sb, wo_sb = w["wqk"], w["wv"], w["wff"], w["wo"]
            bqk_sb, bff_sb, bob_sb = w["bqk"], w["bff"], w["bob"]
            cvec_sb, g1_sb, g2_sb = w["cvec"], w["g1"], w["g2"]
            if l == 0:
                prb("hT0", hT[:])

            # --- qkT = WqkT @ h : rows [q(4 heads)|k(4 heads)], cols T ---
            qk_sb = apool.tile([128, KC, T], F16, tag="qk")
            for n in range(2):
                for m in range(4):
                    ps = psum_mm.tile([128, 512], F32, tag="mm")
                    for k in range(KC):
                        nc.tensor.matmul(
                            ps[:],
                            wqk_sb[:, k, m * 128:(m + 1) * 128],
                            hT[:, k, n * 512:(n + 1) * 512],
                            start=(k == 0), stop=(k == KC - 1),
                        )
                    dst = qk_sb[:, m, n * 512:(n + 1) * 512]
                    if m % 2 == 0:
                        nc.scalar.activation(dst, ps[:], AF.Identity,
                                             bias=bqk_sb[:, m:m + 1])
                    else:
                        nc.vector.tensor_scalar(
                            dst, ps[:], bqk_sb[:, m:m + 1], None,
                            mybir.AluOpType.add)

            # --- v (token-major, with ones column for the softmax denom) ---
            v_sb = apool.tile([128, TC, HC, D + 1], F16, tag="v")
            nc.vector.memset(v_sb[:, :, :, D:D + 1], 1.0)
            for t in range(TC):
                ps = psum_mm.tile([128, HC * D], F32, tag="mm")
                for k in range(KC):
                    nc.tensor.matmul(
                        ps[:],
                        hT[:, k, t * 128:(t + 1) * 128],
                        wv_sb[:, k, :],
                        start=(k == 0), stop=(k == KC - 1),
                    )
                nc.vector.tensor_copy(
                    v_sb[:, t, :, 0:D],
                    ps[:].rearrange("p (h d) -> p h d", h=HC),
                )

            # --- residual base (GPSIMD, off critical path):
            # x = y2*g2 + cvec   (cvec = V-bias + b2[l-1]; l=0: x = h + cvec)
            g2b = g2_sb[:].rearrange("p d -> p () d").broadcast_to([128, TC, DM])
            cvb = cvec_sb[:].rearrange("p d -> p () d").broadcast_to([128, TC, DM])
            if l > 0:
                nc.gpsimd.tensor_mul(h[:], trsrc[:], g2b)
            nc.gpsimd.tensor_add(h[:], h[:], cvb)

            # --- attention (4 local heads, by pairs), token-major output ---
            # After each head pair, its 128 output columns are AllGather-ed so
            # the collective overlaps the next pair's compute.
            a_all = apool.tile([128, TC, HC * D], F16, tag="a_all")
            a_tok = apool.tile([128, TC, DM], F16, tag="a_tok")
            for pair in range(2):
                for hh in (2 * pair, 2 * pair + 1):
                    qT = qk_sb[64 * (hh % 2):64 * (hh % 2) + 64, hh // 2, :]
                    kT = qk_sb[64 * (hh % 2):64 * (hh % 2) + 64, 2 + hh // 2, :]
                    # phase 1: pT[si] = exp(scale * K_si^T Q), diag-masked
                    pts = []
                    for si in range(TC):
                        q0 = si * 128
                        lg = psum_lg.tile([128, T], F32, tag="lg")
                        if q0 < 512:
                            nc.tensor.matmul(lg[:, q0:512], kT[:, q0:q0 + 128],
                                             qT[:, q0:512], start=True, stop=True)
                            nc.tensor.matmul(lg[:, 512:1024], kT[:, q0:q0 + 128],
                                             qT[:, 512:1024], start=True, stop=True)
                        else:
                            nc.tensor.matmul(lg[:, q0:1024], kT[:, q0:q0 + 128],
                                             qT[:, q0:1024], start=True, stop=True)
                        pT = tpool.tile([128, T], F16, tag=f"pT{si}")
                        cols = T - q0
                        nc.scalar.activation(pT[:, 0:cols], lg[:, q0:T], AF.Exp,
                                             scale=float(SCALE))
                        nc.vector.tensor_mul(pT[:, 0:128], pT[:, 0:128], diag_sb[:])
                        pts.append(pT)
                    # phase 2: token-major PV; col D is the softmax denominator
                    for qi in range(TC):
                        pv = psum_pv.tile([128, D + 1], F32, tag="pv")
                        for si in range(qi + 1):
                            off = (qi - si) * 128
                            nc.tensor.matmul(
                                pv[:],
                                pts[si][:, off:off + 128],
                                v_sb[:, si, hh, :],
                                start=(si == 0), stop=(si == qi),
                            )
                        rden = ppool.tile([128, 1], F32, tag="rden")
                        nc.vector.reciprocal(rden[:], pv[:, D:D + 1])
                        nc.scalar.activation(
                            a_all[:, qi, hh * D:(hh + 1) * D], pv[:, 0:D],
                            AF.Identity, scale=rden[:, 0:1])
                # AllGather this pair's 128 columns; assemble + accumulate
                c0, c1 = pair * 128, pair * 128 + 128
                agi = dram.tile([128, TC, 128], F16, tag=f"agi{pair}")
                ago = dram.tile([256, TC, 128], F16, tag=f"ago{pair}")
                nc.sync.dma_start(agi[:], a_all[:, :, c0:c1])
                if no_cc:
                    nc.sync.dma_start(ago[0:128], agi[:])
                    nc.sync.dma_start(ago[128:256], agi[:])
                else:
                    nc.gpsimd.collective_compute(
                        "AllGather", mybir.AluOpType.bypass,
                        replica_groups=[[0, 1], [2, 3], [4, 5], [6, 7]],
                        ins=[agi[:].opt()], outs=[ago[:].opt()],
                    )
                nc.sync.dma_start(a_tok[:, :, c0:c1], ago[0:128])
                nc.sync.dma_start(a_tok[:, :, 256 + c0:256 + c1], ago[128:256])
                nc.vector.tensor_add(
                    h[:, :, c0:c1], h[:, :, c0:c1], a_tok[:, :, c0:c1])
                nc.vector.tensor_add(
                    h[:, :, 256 + c0:256 + c1], h[:, :, 256 + c0:256 + c1],
                    a_tok[:, :, 256 + c0:256 + c1])
            if l == 0:
                prb("a0", a_all[:])

            # --- LN1 -> y1; h1d bounce; x2base on GPSIMD (per half) ---
            y1 = ypool.tile([128, TC, DM], F16, tag="y")
            h1d = dram.tile([TC, 128, DM], F16, tag="h1d")
            g1b = g1_sb[:].rearrange("p d -> p () d").broadcast_to([128, 4, DM])
            bobb = bob_sb[:].rearrange("p d -> p () d").broadcast_to([128, 4, DM])
            for half in range(2):
                t0, t1 = 4 * half, 4 * half + 4
                for t in range(t0, t1):
                    _ln_chunk(nc, ppool, h, y1, t, eps_sb)
                    nc.sync.dma_start(h1d[t], y1[:, t, :])
                hT_transposes(h1T, h1d, half, nc.sync)
                # x2base = y1*g1 + (b1+bo), off critical path
                nc.gpsimd.tensor_mul(h[:, t0:t1, :], y1[:, t0:t1, :], g1b)
                nc.gpsimd.tensor_add(h[:, t0:t1, :], h[:, t0:t1, :], bobb)

            # --- FFN: ffT = relu(Wff'T @ y1 + bff'); o = ffT.T @ Wo ---
            ff_sb = apool.tile([128, FC, T], F16, tag="ff")
            for n in range(2):
                for m in range(FC):
                    ps = psum_mm.tile([128, 512], F32, tag="mm")
                    for k in range(KC):
                        nc.tensor.matmul(
                            ps[:],
                            wff_sb[:, k, m * 128:(m + 1) * 128],
                            h1T[:, k, n * 512:(n + 1) * 512],
                            start=(k == 0), stop=(k == KC - 1),
                        )
                    dst = ff_sb[:, m, n * 512:(n + 1) * 512]
                    if m % 2 == 0:
                        nc.scalar.activation(dst, ps[:], AF.Relu,
                                             bias=bff_sb[:, m:m + 1])
                    else:
                        nc.vector.tensor_scalar(
                            dst, ps[:], bff_sb[:, m:m + 1], 0.0,
                            mybir.AluOpType.add, mybir.AluOpType.max)

            # --- FFN2 + residual + LN2; hd bounce for next layer ---
            y2 = ypool.tile([128, TC, DM], F16, tag="y")
            for t in range(TC):
                ps = psum_mm.tile([128, DM], F32, tag="mm")
                for k in range(FC):
                    nc.tensor.matmul(
                        ps[:],
                        ff_sb[:, k, t * 128:(t + 1) * 128],
                        wo_sb[:, k, :],
                        start=(k == 0), stop=(k == FC - 1),
                    )
                nc.vector.tensor_add(h[:, t, :], h[:, t, :], ps[:])
                _ln_chunk(nc, ppool, h, y2, t, eps_sb)
                if l < L - 1:
                    nc.sync.dma_start(hd[t], y2[:, t, :])
                    if t == 3 or t == 7:
                        hT_transposes(hT, hd, t // 4, nc.sync)
                    if t == 3:
                        wnext = load_weights(l + 1, nc.sync)
            trsrc = y2

        # --- output: h_final = y2*g2 + b2, cast to f32 ---
        b2f_sb = singles.tile([128, DM], F16)
        nc.sync.dma_start(b2f_sb[:], b2f_rep[:])
        ho = singles.tile([128, TC, DM], F32)
        for t in range(TC):
            nc.vector.tensor_mul(h[:, t, :], trsrc[:, t, :], g2_sb[:])
            nc.vector.tensor_add(h[:, t, :], h[:, t, :], b2f_sb[:])
            nc.scalar.copy(ho[:, t, :], h[:, t, :])
            nc.sync.dma_start(out[:, t, :], ho[:, t, :])

    nc.finalize()
    return nc


def _ln_chunk(nc, pool, x, y, t, eps_sb):
    """LayerNorm (no affine) of chunk t: y[:, t, :] = (x_t - mean)/std."""
    stats = pool.tile([128, TC, 6], F32, tag="ln_stats")
    mv = pool.tile([128, TC, 2], F32, tag="ln_mv")
    rstd = pool.tile([128, TC], F32, tag="ln_rstd")
    nc.vector.bn_stats(stats[:, t, :], x[:, t, :])
    nc.vector.bn_aggr(mv[:, t, :], stats[:, t, :])
    nc.scalar.activation(rstd[:, t:t + 1], mv[:, t, 1:2], AF.Sqrt, bias=eps_sb[:])
    nc.vector.reciprocal(rstd[:, t:t + 1], rstd[:, t:t + 1])
    nc.vector.tensor_scalar(
        y[:, t, :], x[:, t, :],
        mv[:, t, 0:1], rstd[:, t:t + 1],
        mybir.AluOpType.subtract, mybir.AluOpType.mult,
    )


_NC_CACHE = {}


def _get_nc(**kw):
    key = tuple(sorted(kw.items()))
    if key not in _NC_CACHE:
        _NC_CACHE[key] = _build(**kw)
    return _NC_CACHE[key]


def _prep_inputs(x, emb, Wqkv, bqkv, Wff, bff, Wo, bo, g1, beta1, g2, beta2):
    """Host-side sharding + LN-affine folding: build the 8 per-core maps."""
    f16 = np.float16
    f32 = np.float32
    emb16 = np.ascontiguousarray((np.asarray(emb) * np.sqrt(f32(DM))).astype(f16))

    p_ = np.arange(T, dtype=f32)[:, None]
    i_ = np.arange(DM, dtype=f32)[None, :]
    rates = 1.0 / np.power(10000.0, 2.0 * np.floor(i_ / 2.0) / DM)
    ang = p_ * rates
    even = (np.arange(DM) % 2) == 0
    pos = np.where(even[None, :], np.sin(ang), np.cos(ang)).astype(f16)
    pos_l = np.ascontiguousarray(pos.reshape(TC, 128, DM).transpose(1, 0, 2))

    Wqkv = np.asarray(Wqkv, f32)
    bqkv = np.asarray(bqkv, f32)
    Wff_ = np.asarray(Wff, f32)
    Wo_ = np.asarray(Wo, f32)
    bff_ = np.asarray(bff, f32)
    bo_ = np.asarray(bo, f32)
    g1_ = np.asarray(g1, f32)
    b1_ = np.asarray(beta1, f32)
    g2_ = np.asarray(g2, f32)
    b2_ = np.asarray(beta2, f32)

    # fold g2[l-1], b2[l-1] into layer l's QKV weights (l >= 1)
    gprev = np.ones((L, DM), f32)
    bprev = np.zeros((L, DM), f32)
    gprev[1:] = g2_[:-1]
    bprev[1:] = b2_[:-1]
    Wqkv_f = Wqkv * gprev[:, :, None]
    bqkv_f = bqkv + np.einsum("ld,ldc->lc", bprev, Wqkv)
    # fold g1[l], b1[l] into Wff
    Wff_f = Wff_ * g1_[:, :, None]
    bff_f = bff_ + np.einsum("ld,ldc->lc", b1_, Wff_)
    bob = (bo_ + b1_).astype(f16)

    Wh = Wqkv_f.reshape(L, DM, H, D, 3)
    bh = bqkv_f.reshape(L, H, D, 3)

    def dm_part(w):  # [L, DM, C] -> [L, 128, KC, C]
        Lx, dm, C = w.shape
        return np.ascontiguousarray(
            w.reshape(Lx, dm // 128, 128, C).transpose(0, 2, 1, 3))

    wff_l = dm_part(Wff_f).astype(f16)
    wo_l = dm_part(Wo_).astype(f16)
    bff_l = np.ascontiguousarray(bff_f.reshape(L, FC, 128).transpose(0, 2, 1))

    def rep(v):  # [L, DM] -> [L, 128, DM] replicated f16
        return np.ascontiguousarray(np.broadcast_to(
            np.asarray(v, f16)[:, None, :], (L, 128, DM)))

    # cvec = V-projection bias (folded) + b2[l-1]: hits the residual directly
    bv_full = bh[:, :, :, 2].reshape(L, DM)
    cvec = bv_full + bprev

    bob_l = rep(bob)
    cvec_l = rep(cvec)
    g1_l = rep(g1_)
    g2_l = rep(g2_)
    b2f = np.ascontiguousarray(
        np.broadcast_to(b2_[L - 1].astype(f16)[None, :], (128, DM)))

    s_i = np.arange(128)[:, None]
    q_i = np.arange(128)[None, :]
    diag = (s_i <= q_i).astype(f16)

    x = np.asarray(x)
    in_maps = []
    for c in range(8):
        seq, half = c // 2, c % 2
        hs = slice(half * HC, half * HC + HC)
        wq = Wh[:, :, hs, :, 0].reshape(L, DM, HC * D)
        wk = Wh[:, :, hs, :, 1].reshape(L, DM, HC * D)
        wqk_c = dm_part(np.concatenate([wq, wk], axis=2)).astype(f16)
        bq = bh[:, hs, :, 0].reshape(L, HC * D)
        bk = bh[:, hs, :, 1].reshape(L, HC * D)
        bqk_c = np.ascontiguousarray(
            np.concatenate([bq, bk], 1).reshape(L, KC, 128).transpose(0, 2, 1)
        ).astype(f32)
        wv_c = dm_part(Wh[:, :, hs, :, 2].reshape(L, DM, HC * D)).astype(f16)

        ids = np.asarray(x[seq], np.int64)
        idx_w = np.ascontiguousarray(
            np.tile(ids.reshape(T // 16, 16).T.astype(np.int16), (8, 1)))

        in_maps.append({
            "emb16": emb16, "idxs": idx_w, "pos": pos_l,
            "wqk": wqk_c, "bqk": bqk_c, "wv": wv_c,
            "wff": wff_l, "bff": bff_l, "wo": wo_l, "bob_rep": bob_l,
            "cvec_rep": cvec_l,
            "g1_rep": g1_l, "g2_rep": g2_l, "b2f_rep": b2f,
            "diagm": diag,
        })
    return in_maps


def kernel(**inputs) -> np.ndarray:
    nc = _get_nc()
    in_maps = _prep_inputs(**inputs)
    res = run_bass_kernel_spmd(nc, in_maps, core_ids=list(range(8)))
    outs = []
    for seq in range(N):
        o = res.results[2 * seq]["out"]  # [128, TC, DM], token t = c*128+p
        outs.append(o.transpose(1, 0, 2).reshape(T, DM))
    return np.stack(outs).astype(np.float32)
